# revision 34
# baseline (speedup 1.0000x reference)
"""Multi-head attention (B=2, S=2048, D=1024, H=16, dk=dv=64) on 8 TRN2 cores.

Sharding: core c -> batch b = c % 2, head-group g = c // 2 (heads 4g..4g+3).
Each core computes its 4 heads' attention for one batch plus the partial
output projection; the host sums the 4 partials per batch and adds bo plus
the (constant) V-bias term SCALE*bv@Wo -- softmax weights sum to 1, so the
V-bias contributes a constant vector that never needs to touch the device.

Device pipeline (weights/activations fp16, exp/ctx path bf16, all PSUM
accumulation fp32 -- measured end-to-end rel err ~2.8e-3 vs the 2e-2 gate).
Softmax exp alternates per t-tile between the scalar engine (exact,
table-based) and the vector engine (Schraudolph int16 fast-exp, one fused
op); superstep structure keeps the PE's weight-switch drain exposure low:

  1. Prologue: inputs stream in 1MB quarters in need order (weights
     host-packed to one 4KB DMA element per partition row); a burst of
     throwaway matmuls during the DMA wait trips the HAM clock ramp early.
     Only K-proj(pair0, cc0) + Q-proj(pair0, b0) run inline; all other
     K/Q projections are fillers inside the first two attention segments.
  2. V proj in NATURAL orientation: lhsT = vt chunk (D on partitions),
     rhs = wv -> vw[t, 4*(dv+1)] directly, no PE transposes. The 65th column
     per head holds 1/SCALE (softmax denominator, pre-scaled). Fillers in
     the first attention segment.
  3. Attention per (pair, block): 8 supersteps x 2 t-tiles:
     [ctx kc,kc+1: 4 matmuls] [2 fillers] [scores tt,tt+1: 2 row-group
     pairs + exp]. ctx trails scores by LAG=4 t-tiles. Fillers: deferred
     K/Q proj pieces, next block's Q proj, previous block's out proj,
     normalize.
  4. Normalize: reciprocal on the [1,512] denominator row (ACT copy ->
     DVE recip), gpsimd partition-broadcast, one DVE multiply into
     persistent ctxT. Out proj [s,D] partials with ACT eviction, DMA'd out.
  5. Last block runs pair 1 first; its normalize hides in the pair-0
     segment. Tail: striped normalize (256-col halves) + out proj groups
     ordered so PSUM tags are reused 2 groups apart, DMA per 512-col half.
"""
import os
import sys

sys.path.insert(0, "/opt/trn_rl_repo")
os.environ.setdefault("JAX_PLATFORMS", "axon,cpu")

from collections import deque
from contextlib import ExitStack

import numpy as np

import concourse.bacc as bacc
import concourse.tile as tile
from concourse import mybir
from concourse.bass_utils import run_bass_kernel_spmd

FP32 = mybir.dt.float32
FP16 = mybir.dt.float16
BF16 = mybir.dt.bfloat16
I16 = mybir.dt.int16

B, S, D = 2, 2048, 1024
H, DK, DV = 16, 64, 64
N_CORES = 8
HPC = H // (N_CORES // B)  # heads per core = 4
P = 128
SBLK = 512                # s-block (free dim of scores matmuls)
NBLK = S // SBLK          # 4
NTT = S // P              # 16 t-tiles
NDC = D // P              # 8 contraction chunks
NV = HPC * (DV + 1)       # 260
SCALE = 1.0 / (DK * 2.0)  # folded into the softmax denominator on device
LAG = 4                   # ctx trails exp by LAG t-tile steps

# Schraudolph fast-exp, bf16 flavor: i16 = round(x * EXP_A16 + EXP_B16) is
# the bit pattern of bf16 ~= exp(x) * (1 + eps), |eps| < ~3% sawtooth. The
# global factor cancels in softmax; measured end-to-end rel err ~2.1e-3.
EXP_A16 = float((1 << 23) / np.log(2.0) / 65536.0)
EXP_B16 = float((127 * (1 << 23) - 486408) / 65536.0)
# steps whose exp runs on the DVE instead of ACT (50/50 split; the DVE no
# longer carries the normalize broadcast/reciprocal bulk or out evictions)
def _exp_on_dve(k):
    return k % 2 == 1


def _build_nc():
    nc = bacc.Bacc("TRN2", target_bir_lowering=False, debug=False,
                   num_devices=N_CORES)
    d = {}
    for name, shape in [
        ("qt", [D, S]), ("kt", [D, S]), ("vt", [D, S]),
        # weights host-packed to the SBUF layout: one contiguous 4KB DMA
        # element per partition row (vs 512B strided -- 4x fewer packets)
        ("wq", [P, NDC, 2 * P]), ("wk", [P, NDC, 2 * P]),
        ("wv", [P, NDC, 2 * P]), ("wo", [P, 2, D]),
    ]:
        d[name] = nc.dram_tensor(name, shape, FP16, kind="ExternalInput").ap()
    d["bqk"] = nc.dram_tensor("bqk", [P, 4], FP32, kind="ExternalInput").ap()
    out_d = nc.dram_tensor("out", [S, D], FP16, kind="ExternalOutput").ap()
    xt_view = {
        n: d[n].rearrange("(dc p) s -> p dc s", p=P)
        for n in ("qt", "kt", "vt")
    }

    with tile.TileContext(nc) as tc, ExitStack() as ctx:
        const = ctx.enter_context(tc.tile_pool(name="const", bufs=1))
        wpool = ctx.enter_context(tc.tile_pool(name="wpool", bufs=1))
        xtp = ctx.enter_context(tc.tile_pool(name="xtp", bufs=4))
        projp = ctx.enter_context(tc.tile_pool(name="projp", bufs=1))
        expp = ctx.enter_context(tc.tile_pool(name="expp", bufs=1))
        ctxp = ctx.enter_context(tc.tile_pool(name="ctxp", bufs=1))
        outp = ctx.enter_context(tc.tile_pool(name="outp", bufs=2))
        smallp = ctx.enter_context(tc.tile_pool(name="smallp", bufs=2))
        psum = ctx.enter_context(tc.tile_pool(name="psum", bufs=1, space="PSUM"))

        # ---- constants / weights (wk first: K projection starts the kernel) ----
        def load_w(sb, name):
            nc.sync.dma_start(sb[:], d[name])

        wk_sb = wpool.tile([P, NDC, 2 * P], FP16)
        load_w(wk_sb, "wk")
        bqk = const.tile([P, 4], FP32)
        nc.sync.dma_start(bqk[:], d["bqk"])
        wq_sb = wpool.tile([P, NDC, 2 * P], FP16)
        wv_sb = wpool.tile([P, NDC, 2 * P], FP16)
        wo_sb = wpool.tile([P, 2, D], FP16)

        # ---- persistent activation tiles ----
        qwt = [projp.tile([P, S], FP16, tag=f"qwt{p_}", name=f"qwt{p_}") for p_ in range(2)]
        kwt = [projp.tile([P, S], FP16, tag=f"kwt{p_}", name=f"kwt{p_}") for p_ in range(2)]
        vw = projp.tile([P, NTT, NV], BF16, tag="vw")
        # softmax-denominator column (once, strided over the 65-wide head
        # slots). Value 1/SCALE pre-scales the denominator so normalize is a
        # plain reciprocal+multiply (no separate scaling op).
        for hh in range(HPC):
            nc.vector.memset(vw[:, :, hh * (DV + 1) + DV], 1.0 / SCALE)
        ctx_t = [ctxp.tile([P, S], FP16, tag=f"ctx{p_}", name=f"ctx{p_}") for p_ in range(2)]

        def load_chunk(name, col0, width=SBLK, tag=None, bufs=1):
            xt = xtp.tile([P, NDC, width], FP16, tag=tag or f"xt{width}",
                          name="xt", bufs=bufs)
            nc.sync.dma_start(xt[:], xt_view[name][:, :, col0:col0 + width])
            return xt

        def proj_qk_pair(xt, off, w_sb, dst, bias_col, ci, pair, tag):
            """Project one head-pair of a 512-slice into dst[pair][:, ci*SBLK:...]."""
            pq = psum.tile([P, SBLK], FP32, tag=tag, name="pq")
            for dc in range(NDC):
                nc.tensor.matmul(pq[:], lhsT=w_sb[:, dc, pair * P:(pair + 1) * P],
                                 rhs=xt[:, dc, off:off + SBLK],
                                 start=(dc == 0), stop=(dc == NDC - 1))
            nc.scalar.activation(dst[pair][:, ci * SBLK:(ci + 1) * SBLK], pq[:],
                                 mybir.ActivationFunctionType.Identity,
                                 bias=bqk[:, bias_col + pair:bias_col + pair + 1])

        def proj_qk_piece(xt, off, w_sb, dst, bias_col, ci, pair, dc_range, pq_holder):
            if dc_range[0] == 0:
                pq_holder[pair] = psum.tile([P, SBLK], FP32, tag="pj", name="pq")
            pq = pq_holder[pair]
            for dc in dc_range:
                nc.tensor.matmul(pq[:], lhsT=w_sb[:, dc, pair * P:(pair + 1) * P],
                                 rhs=xt[:, dc, off:off + SBLK],
                                 start=(dc == 0), stop=(dc == NDC - 1))
            if dc_range[-1] == NDC - 1:
                nc.scalar.activation(dst[pair][:, ci * SBLK:(ci + 1) * SBLK], pq[:],
                                     mybir.ActivationFunctionType.Identity,
                                     bias=bqk[:, bias_col + pair:bias_col + pair + 1])

        # V proj, natural orientation: one t-tile per call (8 matmuls, free=256).
        # vp holds 2 t-tiles per PSUM bank; evicted per t-tile by the DVE.
        vp_holder = [None]

        def proj_v_tt(vt_chunk, tt):
            par = tt % 2
            if par == 0:
                vp_holder[0] = psum.tile([P, 2, 2 * P], FP32, tag="po", name="vp")
            vp = vp_holder[0]
            off = (tt * P) % SBLK
            for dc in range(NDC):
                nc.tensor.matmul(vp[:, par, :],
                                 lhsT=vt_chunk[:, dc, off:off + P],
                                 rhs=wv_sb[:, dc, :],
                                 start=(dc == 0), stop=(dc == NDC - 1))
            # evict into the 65-wide head slots (dv 0:64 of each slot)
            nc.vector.tensor_copy(
                vw[:, tt, :].rearrange("p (h v) -> p h v", v=DV + 1)[:, :, 0:DV],
                vp[:, par, :].rearrange("p (h v) -> p h v", h=HPC))

        def attn_normalize_hp(pair, b, ct, hp, c0=0, cw=SBLK):
            # ctx = ct[0:64] / ct[64] row-broadcast, one head (the ones-column
            # value 1/SCALE already folded SCALE into the denominator row).
            # Reciprocal FIRST on the [1,cw] row (DVE cost goes by free size,
            # so the row op costs the same as the [64,cw] one), then the idle
            # gpsimd broadcasts the small row off the DVE path.
            rcr = smallp.tile([1, SBLK], FP32, tag="rcr")
            nc.scalar.activation(rcr[:, 0:cw], ct[hp][DV:DV + 1, c0:c0 + cw],
                                 mybir.ActivationFunctionType.Identity)
            rcp1 = smallp.tile([1, SBLK], FP32, tag="rcp1")
            nc.vector.reciprocal_approx_fast(rcp1[:, 0:cw], rcr[:, 0:cw])
            rbc = smallp.tile([DV, SBLK], FP32, tag="rbc")
            nc.gpsimd.partition_broadcast(rbc[:, 0:cw], rcp1[:, 0:cw],
                                          channels=DV)
            nc.vector.tensor_mul(
                ctx_t[pair][hp * DV:(hp + 1) * DV,
                            b * SBLK + c0:b * SBLK + c0 + cw],
                ct[hp][0:DV, c0:c0 + cw], rbc[:, 0:cw])

        def norm_fillers(pair, b, ct):
            return [lambda h=hp: attn_normalize_hp(pair, b, ct, h)
                    for hp in range(2)]

        def norm_striped(pair, b, ct):
            # last-block tail variant: normalize in 256-wide halves so the
            # tail out-projection of the first half starts ~1.3us earlier
            return [lambda h=hp, c=half * (SBLK // 2):
                    attn_normalize_hp(pair, b, ct, h, c, SBLK // 2)
                    for half in range(2) for hp in range(2)]

        def out_proj_st(b, st, tag="po"):
            # one 128-row slice of the output: both 512-wide halves of D,
            # one batched row DMA out. Eviction on the scalar engine (the
            # DVE carries half the exps; ACT has the slack here).
            off = b * SBLK + st * P
            ob = outp.tile([P, D], FP16, tag="ob")
            for nh in range(2):
                po = psum.tile([P, SBLK], FP32, tag=tag, name="po")
                for jc in range(2):
                    nc.tensor.matmul(po[:],
                                     lhsT=ctx_t[jc][:, off:off + P],
                                     rhs=wo_sb[:, jc, nh * SBLK:(nh + 1) * SBLK],
                                     start=(jc == 0), stop=(jc == 1))
                nc.scalar.activation(ob[:, nh * SBLK:(nh + 1) * SBLK], po[:],
                                     mybir.ActivationFunctionType.Identity)
            nc.sync.dma_start(out_d[off:off + P, :], ob[:])

        def tail_out(b):
            # final block's out projection. Group order follows the striped
            # normalize (st0/st1 only need the first 256-col half); each PSUM
            # tag (po/pj by st parity) is reused two matmul-groups later so
            # the ACT eviction fully overlaps; DMA fires per 512-wide half.
            obs = [outp.tile([P, D], FP16, tag="obt", name="obt", bufs=4)
                   for _ in range(4)]
            for st, nh in [(0, 0), (1, 0), (0, 1), (1, 1),
                           (2, 0), (3, 0), (2, 1), (3, 1)]:
                off = b * SBLK + st * P
                po = psum.tile([P, SBLK], FP32,
                               tag="po" if st % 2 == 0 else "pj", name="po")
                # jc=1 (pair 1) first: that half of ctx_t was normalized
                # during the preceding segment, so the group's first matmul
                # issues in the shadow of the pair-0 normalize chain
                for jc in (1, 0):
                    nc.tensor.matmul(
                        po[:], lhsT=ctx_t[jc][:, off:off + P],
                        rhs=wo_sb[:, jc, nh * SBLK:(nh + 1) * SBLK],
                        start=(jc == 1), stop=(jc == 0))
                nc.scalar.activation(obs[st][:, nh * SBLK:(nh + 1) * SBLK],
                                     po[:],
                                     mybir.ActivationFunctionType.Identity)
                nc.sync.dma_start(
                    out_d[off:off + P, nh * SBLK:(nh + 1) * SBLK],
                    obs[st][:, nh * SBLK:(nh + 1) * SBLK])

        # ---- prologue: stream inputs in 1MB quarters, in need order. Only
        # the two projections gating the first scores (K pair0 cc0, Q pair0
        # block0) run inline; every other projection becomes a filler inside
        # the first two attention segments. ----
        # first kt quarter in two dc-halves: the cc0 projection's first
        # matmul only waits on the 512KB dc0-3 half
        kt0 = xtp.tile([P, NDC, SBLK], FP16, tag="ktq", name="xt", bufs=4)
        nc.sync.dma_start(kt0[:, 0:4, :], xt_view["kt"][:, 0:4, 0:SBLK])
        # PE warm-up: ~20 throwaway matmuls on a memset tile during the
        # dead DMA window -- trips the HAM clock ramp (half rate until ~5-9us
        # after first PE activity) before the real projections start
        warm = const.tile([P, SBLK], FP16)
        nc.vector.memset(warm[:], 0.0)
        pw = psum.tile([P, SBLK], FP32, tag="pj", name="pw")
        for _ in range(30):
            nc.tensor.matmul(pw[:], lhsT=warm[:, 0:P], rhs=warm[:],
                             start=True, stop=True)
        nc.sync.dma_start(kt0[:, 4:8, :], xt_view["kt"][:, 4:8, 0:SBLK])
        ktq = [kt0, load_chunk("kt", SBLK, tag="ktq", bufs=4)]
        qt0 = load_chunk("qt", 0, tag="qt0")
        # wv before wq: the first V-proj filler (segment 0, superstep 0)
        # otherwise stalls ~2.4us on the late wv transfer
        load_w(wv_sb, "wv")
        load_w(wq_sb, "wq")
        vts = [load_chunk("vt", 0, tag="vtq", bufs=4)]
        ktq.append(load_chunk("kt", 2 * SBLK, tag="ktq", bufs=4))
        vts.append(load_chunk("vt", SBLK, tag="vtq", bufs=4))
        ktq.append(load_chunk("kt", 3 * SBLK, tag="ktq", bufs=4))
        vts.append(load_chunk("vt", 2 * SBLK, tag="vtq", bufs=4))
        vts.append(load_chunk("vt", 3 * SBLK, tag="vtq", bufs=4))
        qt_rest = [load_chunk("qt", SBLK, width=3 * SBLK, tag="qtr")]
        load_w(wo_sb, "wo")
        proj_qk_pair(ktq[0], 0, wk_sb, kwt, 2, 0, 0, "pj")
        # keep the PE warm through the qt0/wq DMA wait so HAM doesn't drop back
        pw2 = psum.tile([P, SBLK], FP32, tag="po", name="pw")
        for _ in range(16):
            nc.tensor.matmul(pw2[:], lhsT=warm[:, 0:P], rhs=warm[:],
                             start=True, stop=True)
        proj_qk_pair(qt0, 0, wq_sb, qwt, 0, 0, 0, "po")

        # deferred projection pieces (half a head-pair projection each);
        # pieces of one (cc, pair) group stay contiguous so only one "pj"
        # PSUM holder is ever open
        _holders = {}

        def kq_piece(cc, pair, half):
            h = _holders.setdefault(("k", cc, pair), [None, None])
            dcs = (0, 1, 2, 3) if half == 0 else (4, 5, 6, 7)
            return lambda: proj_qk_piece(ktq[cc], 0, wk_sb, kwt, 2, cc, pair,
                                         dcs, h)

        def q0_piece(pair, half):
            h = _holders.setdefault(("q0", pair), [None, None])
            dcs = (0, 1, 2, 3) if half == 0 else (4, 5, 6, 7)
            return lambda: proj_qk_piece(qt0, 0, wq_sb, qwt, 0, 0, pair,
                                         dcs, h)

        def pair2(f1, f2):
            return lambda: (f1(), f2())

        def interleave(a, bl):
            out = []
            for i in range(max(len(a), len(bl))):
                if i < len(a):
                    out.append(a[i])
                if i < len(bl):
                    out.append(bl[i])
            return out

        # ---- attention: (pair, block) segments of 16 t-tile steps ----
        def attn_segment(pair, b, ct, fillers):
            """Superstep = 2 t-tiles: [ctx kc,kc+1] [2 fillers] [scores+exp
            tt,tt+1]. Grouping same-type matmuls halves the PE weight-switch
            drain exposure vs the 1-tile step."""
            exs = {}
            for k2 in range(0, NTT + LAG, 2):
                for kc in (k2 - LAG, k2 - LAG + 1):
                    if kc >= 0:
                        ex = exs.pop(kc)
                        for hp in range(2):
                            hh = 2 * pair + hp
                            nc.tensor.matmul(
                                ct[hp][:],
                                lhsT=vw[:, kc, hh * (DV + 1):(hh + 1) * (DV + 1)],
                                rhs=ex[:, hp, :],
                                start=(kc == 0), stop=(kc == NTT - 1))
                for _ in range(2):
                    if fillers:
                        fillers.pop(0)()
                for tt in (k2, k2 + 1):
                    if tt >= NTT:
                        continue
                    sc = psum.tile([P, 2, SBLK], FP32, tag="sc", name="sc", bufs=2)
                    for hp in range(2):
                        lo, hi = hp * DK, (hp + 1) * DK
                        nc.tensor.matmul(
                            sc[:, hp, :],
                            lhsT=kwt[pair][lo:hi, tt * P:(tt + 1) * P],
                            rhs=qwt[pair][lo:hi, b * SBLK:(b + 1) * SBLK],
                            start=True, stop=True)
                    ex = expp.tile([P, 2, SBLK], BF16, tag="exp", name="ex",
                                   bufs=LAG + 3)
                    if _exp_on_dve(tt):
                        # fused fast-exp: int16(x*A + B) is bf16 ~= exp(x)
                        nc.vector.tensor_scalar(
                            ex[:].bitcast(I16), sc[:], EXP_A16, EXP_B16,
                            mybir.AluOpType.mult, mybir.AluOpType.add)
                    else:
                        nc.scalar.activation(ex[:], sc[:],
                                             mybir.ActivationFunctionType.Exp)
                    exs[tt] = ex

        def attn_alloc():
            return [psum.tile([DV + 1, SBLK], FP32, tag=f"ct{hp}", name=f"ct{hp}")
                    for hp in range(2)]

        def v_filler(tt):
            return lambda: proj_v_tt(vts[tt // 4], tt)

        prev_norm = []    # second-pair normalize deferred into the next block
        for b in range(NBLK):
            have_next = b + 1 < NBLK
            # last block runs pair 1 FIRST so its normalize hides inside the
            # pair-0 segment; only pair 0's normalize gates the tail.
            first, second = (0, 1) if have_next else (1, 0)
            fill0 = list(prev_norm)
            prev_norm = []
            if b == 0:
                # V projection (consumed at step tt+LAG) woven with the
                # deferred K/Q projections; kq(cc, pair0) must land before
                # scores step 4*cc, pair1/q0 pieces before the next segment
                v = [v_filler(tt) for tt in range(NTT)]
                fill0 += [
                    kq_piece(1, 0, 0),
                    pair2(kq_piece(1, 0, 1), v[0]),
                    pair2(v[1], kq_piece(0, 1, 0)),
                    pair2(v[2], kq_piece(0, 1, 1)),
                    v[3],
                    pair2(kq_piece(2, 0, 0), v[4]),
                    pair2(kq_piece(2, 0, 1), v[5]),
                    v[6],
                    pair2(v[7], q0_piece(1, 0)),
                    pair2(q0_piece(1, 1), v[8]),
                    pair2(kq_piece(3, 0, 0), v[9]),
                    pair2(kq_piece(3, 0, 1), v[10]),
                    v[11], v[12], v[13], v[14], v[15],
                ]
            else:
                fill0 += [lambda s=st, bb=b: out_proj_st(bb - 1, s)
                          for st in range(2)]
            ct = attn_alloc()
            attn_segment(first, b, ct, fill0)

            fill1 = norm_fillers(first, b, ct)
            pp = []
            if have_next:
                holder = [None, None]
                for pair_ in range(2):
                    for dcs in ([0, 1], [2, 3], [4, 5], [6, 7]):
                        pp.append(lambda p=pair_, r=tuple(dcs), h=holder, bb=b:
                                  proj_qk_piece(qt_rest[0], bb * SBLK,
                                                wq_sb, qwt, 0, bb + 1, p, r, h))
            op = []
            if b > 0:
                op += [lambda s=st, bb=b: out_proj_st(bb - 1, s)
                       for st in range(2, 4)]
            ct1 = attn_alloc()
            if b == 0:
                # pair1's remaining K projection, then next-block Q: each
                # (cc,pair) piece group contiguous (single open "pj" holder)
                k1 = [kq_piece(c, 1, h) for c in (1, 2, 3) for h in (0, 1)]
                fill_second = fill1 + k1 + pp
            else:
                fill_second = fill1 + interleave(pp, op)
            attn_segment(second, b, ct1, fill_second)
            prev_norm = (norm_striped if not have_next else
                         norm_fillers)(second, b, ct1)
        for f in prev_norm:
            f()
        tail_out(NBLK - 1)

    nc.compile()
    return nc


_NC_CACHE = None


def _get_nc():
    global _NC_CACHE
    if _NC_CACHE is None:
        _NC_CACHE = _build_nc()
    return _NC_CACHE


def kernel(Q, K, V, Wq, bq, Wk, bk, Wv, bv, Wo, bo, _trace=False, _trace_kwargs=None):
    nc = _get_nc()
    f16 = np.float16
    qt_h = [np.ascontiguousarray(np.asarray(Q[b]).T.astype(f16)) for b in range(B)]
    kt_h = [np.ascontiguousarray(np.asarray(K[b]).T.astype(f16)) for b in range(B)]
    vt_h = [np.ascontiguousarray(np.asarray(V[b]).T.astype(f16)) for b in range(B)]

    in_maps = []
    for c in range(N_CORES):
        b, g = c % B, c // B
        hs = list(range(g * HPC, (g + 1) * HPC))
        wq_p = np.concatenate([Wq[h] for h in hs], axis=1)
        wk_p = np.concatenate([Wk[h] for h in hs], axis=1)
        wv_p = np.concatenate([Wv[h] for h in hs], axis=1)
        bqk_p = np.stack([
            np.concatenate([bq[hs[0]], bq[hs[1]]]),
            np.concatenate([bq[hs[2]], bq[hs[3]]]),
            np.concatenate([bk[hs[0]], bk[hs[1]]]),
            np.concatenate([bk[hs[2]], bk[hs[3]]]),
        ], axis=1)
        def pack(w, groups):
            # [groups*128, m] -> [128, groups, m]: one contiguous DMA element
            # per SBUF partition row
            return np.ascontiguousarray(
                w.reshape(groups, P, w.shape[1]).transpose(1, 0, 2).astype(f16))

        in_maps.append({
            "qt": qt_h[b], "kt": kt_h[b], "vt": vt_h[b],
            "wq": pack(wq_p, NDC),
            "wk": pack(wk_p, NDC),
            "wv": pack(wv_p, NDC),
            "bqk": np.ascontiguousarray(bqk_p.astype(np.float32)),
            "wo": pack(Wo[g * HPC * DV:(g + 1) * HPC * DV], 2),
        })

    kw = {}
    if _trace:
        kw = dict(trace=True, **(_trace_kwargs or {}))
    res = run_bass_kernel_spmd(nc, in_maps, core_ids=list(range(N_CORES)), **kw)

    out = np.zeros((B, S, D), dtype=np.float32)
    for c in range(N_CORES):
        out[c % B] += res.results[c]["out"].astype(np.float32)
    # host-side constant terms: output bias + V-bias (softmax weights sum to 1,
    # so the V bias contributes SCALE * bv @ Wo, constant over (b, s))
    out += bo[None, None, :] + (SCALE * bv.reshape(-1)) @ Wo
    if _trace:
        return out, res
    return out

